# revision 8
# baseline (speedup 1.0000x reference)
"""Trainium2 Bass kernel for 3D conv: x[4,16,64,64,64] * w[16,16,3,3,3] + bias, pad=1.

Strategy (8 cores): per batch, the 64 output d-planes are split between a core
pair as 5 full-width d-blocks each (A: d[0,30), B: d[34,64)) plus one shared
half-width block for d[30,34) (A computes w[0,32), B computes w[32,64)), so
every core streams 5 full planes + 1 half plane instead of 6 full planes.
Per core, a "banded weight" matmul decomposition:
  - contraction K = (cin=16) x (d-window=8) = 128 partitions
  - output    M = (d_sub=6 outputs) x (cout=16) = 96 partitions
  - the 9 (kh,kw) taps are free-dim shifts over a zero-padded (h,w) plane
  - lhsT[(cin,dw), (ds,co)] = W[co,cin,dw-ds,kh,kw] for 0<=dw-ds<=2 (banded)
Matmuls in bf16 (fp32 psum accumulation); Ldweights fully hidden at steady
state. Bias is fused into the PSUM->SBUF extraction copy on the scalar engine.
DMA: x shards stream on the sync HWDGE ring (block-0's first rows lead),
weights/bias load on the scalar ring in parallel; full-block outputs store via
gpsimd SWDGE; the final half-block stores ride the then-idle sync ring.
"""

import os
from contextlib import ExitStack

import ml_dtypes
import numpy as np

import concourse.bass as bass
import concourse.mybir as mybir
import concourse.tile as tile
from concourse.bass_utils import run_bass_kernel_spmd

_MMDT = mybir.dt.bfloat16
_IODT = mybir.dt.bfloat16
_NPDT = ml_dtypes.bfloat16

B, CIN, COUT, S = 4, 16, 16, 64
HP = S + 2                 # padded rows: 66 (zero rows 0 and 65)
WP = S + 1                 # row stride 65: each row's right pad IS the next
                           # row's left pad (col 0 of every row is zero)
PLANE = HP * WP + 1        # 4291; +1 so the (kh,kw)=(+1,+1) corner read of
                           # the last valid element lands on a host zero
MARGIN = 68                # free-dim margin so shifted reads stay in-bounds
DSUB = 6                   # d outputs per full block
NDW = 8                    # d-window planes (DSUB + 2 halo)
NFULL = 5                  # full-width d-blocks per core (d outputs 0..29)
SHARD_D = 32               # padded d planes per shard (windows span 5 blocks)
CROWS = 7                  # padded h-rows per psum chunk (7*66=462 <= 512)
OBW = CROWS * S            # 448 output cols per full chunk
P0 = 9 * WP                # 585: block-0 lead piece (chunk 0 reads rows [0,9))
XSPLIT = 24 * WP           # piece-A/piece-B split: rows [0,24) / [24,66)

# half-width block (d outputs 30..33, one w-half per core)
SH = 32                    # output cols per row in the half block
WPH = SH + 2               # half-plane row stride: [halo/pad, 32 outputs, halo]
PLANE_H = HP * WPH + 1     # 2245
OBWH = CROWS * SH          # 224
MVH = 64                   # stored partitions: (ds=4 used of 6) x 16

_nc_cache = None
LAST_RESULT = None         # BassKernelResults of the most recent run (for test.py)


def _strip_implied_waits(nc):
    """Remove semaphore waits that are transitively implied by another wait on
    the same instruction.

    Tile's add_semaphores emits the full non-transitive closure, so a matmul
    whose psum slot was last touched by (PE writes -> ACT read -> DVE memset)
    carries three waits — but walrus only supports a single sync-wait on a
    Matmult (fp32r matmuls are self-loading, and the wait rides the LDW
    struct). A wait (s >= v) is provably redundant if another wait on the
    same instruction targets a producer whose completion already implies
    (s >= v). We replay the scheduled instruction stream with vector clocks
    to compute each semaphore event's implied clock, then drop implied waits.

    In-order completion is assumed per compute-engine queue but NOT for DMA
    instructions (SDMA engines complete packets out of order), so DMA clocks
    only carry their own waits + update.
    """
    sem_count = {}
    sem_events = {}        # sem id -> list of (value_after, clock dict)
    engine_clock = {}
    engine_self = {}       # engine -> {sem id -> updates issued by that engine}

    def join(a, b):
        for k, v in b.items():
            if a.get(k, -1) < v:
                a[k] = v

    def snapshot(sid, val):
        for value_after, clk in sem_events.get(sid, ()):
            if value_after >= val:
                return clk
        return None

    for block in nc.m.functions[0].blocks:
        for inst in block.instructions:
            si = inst.sync_info
            if si is None:
                continue
            eng0 = str(inst.engine)
            is_dma0 = type(inst).__name__ in ("InstDMACopy", "InstDMATranspose")
            is_serial = (not is_dma0 and type(inst).__name__ not in
                         ("InstMatmult", "InstDrain", "InstEventSemaphore"))
            waits = list(si.on_wait)
            if is_serial and len(waits) > 1:
                # serial engines execute in order: a wait on the engine's own
                # completion semaphore for a value its predecessors already
                # produce is a no-op.
                own = engine_self.get(eng0, {})
                kept = [w for w in waits
                        if not (w.wait_mode == "sem-ge-imm"
                                and own.get(w.id, 0) >= w.wait_value)]
                if len(kept) < len(waits):
                    si.on_wait = kept
                    waits = kept
            snaps = []
            for w in waits:
                snaps.append(snapshot(w.id, w.wait_value)
                             if w.wait_mode == "sem-ge-imm" else None)
            if len(waits) > 1:
                keep = []
                for i, w in enumerate(waits):
                    if w.wait_mode != "sem-ge-imm":
                        keep.append(w)
                        continue
                    implied = False
                    for j, other in enumerate(waits):
                        if i == j or snaps[j] is None:
                            continue
                        if snaps[j].get(w.id, -1) >= w.wait_value:
                            implied = True
                            break
                    if not implied:
                        keep.append(w)
                if len(keep) < len(waits):
                    si.on_wait = keep
                    waits = keep
                    snaps = [snapshot(w.id, w.wait_value)
                             if w.wait_mode == "sem-ge-imm" else None
                             for w in waits]

            clk = {}
            for s in snaps:
                if s is not None:
                    join(clk, s)
            eng = str(inst.engine)
            is_dma = type(inst).__name__ in ("InstDMACopy", "InstDMATranspose")
            if not is_dma and eng in engine_clock:
                join(clk, engine_clock[eng])
            for u in si.on_update:
                if u.update_mode == "sem-add-imm":
                    sem_count[u.id] = sem_count.get(u.id, 0) + u.update_value
                elif u.update_mode == "sem-inc":
                    sem_count[u.id] = sem_count.get(u.id, 0) + 1
                else:
                    continue
                clk[u.id] = max(clk.get(u.id, 0), sem_count[u.id])
                sem_events.setdefault(u.id, []).append((sem_count[u.id], clk))
                if not is_dma:
                    es = engine_self.setdefault(eng, {})
                    es[u.id] = sem_count[u.id]
            if not is_dma:
                engine_clock[eng] = clk


def _pair_waits_to_ldw(nc):
    """A bf16 matmul is an Ldweights+Matmult pair; each half carries one
    sync-wait slot. If the Matmult holds two waits and its paired Ldweights
    holds none, move one over (gating the pair earlier is safe)."""
    import bass_rust
    for block in nc.m.functions[0].blocks:
        insts = list(block.instructions)
        for idx, inst in enumerate(insts):
            si = inst.sync_info
            if (type(inst).__name__ != "InstMatmult" or not si
                    or len(si.on_wait) <= 1 or idx == 0):
                continue
            prev = insts[idx - 1]
            if type(prev).__name__ != "InstLdweights":
                continue
            psi = prev.sync_info
            if psi is not None and psi.on_wait:
                continue
            waits = list(si.on_wait)
            moved = waits.pop()
            if psi is None:
                prev.sync_info = bass_rust.SyncInfo(on_wait=[moved],
                                                    on_update=[])
            else:
                psi.on_wait = [moved]
            si.on_wait = waits


def _push_waits_earlier(nc):
    """For a DMACopy still carrying >1 waits (e.g. a DMAHW lane-ordering wait
    plus a data wait), move the extras onto an earlier zero-wait instruction
    of the same engine queue. Satisfying a wait earlier in the queue is
    strictly more conservative — provided the instruction that produces the
    awaited semaphore value is not itself issued later on that queue (which
    would deadlock). Producers are located with a semaphore replay."""
    for block in nc.m.functions[0].blocks:
        insts = list(block.instructions)
        # replay semaphore counts to locate each (sem, value)'s producer idx
        counts = {}
        events = {}  # sem id -> list of (value_after, idx)
        for idx, inst in enumerate(insts):
            si = inst.sync_info
            if not si:
                continue
            for u in si.on_update:
                inc = u.update_value if u.update_mode == "sem-add-imm" else (
                    1 if u.update_mode == "sem-inc" else 0)
                if not inc:
                    continue
                counts[u.id] = counts.get(u.id, 0) + inc
                events.setdefault(u.id, []).append((counts[u.id], idx))

        def producer_idx(sid, val):
            for value_after, idx in events.get(sid, ()):
                if value_after >= val:
                    return idx
            return None

        for idx, inst in enumerate(insts):
            si = inst.sync_info
            if (type(inst).__name__ != "InstDMACopy"
                    or not si or len(si.on_wait) <= 1):
                continue
            waits = list(si.on_wait)
            keep = [w for w in waits if not w.ant_name.startswith("DMAHW")]
            extras = [w for w in waits if w.ant_name.startswith("DMAHW")]
            if len(keep) + min(1, len(extras)) <= 1:
                continue
            eng = str(inst.engine)
            for eidx in range(idx - 1, -1, -1):
                if len(keep) + len(extras) <= 1:
                    break
                earlier = insts[eidx]
                if str(earlier.engine) != eng:
                    continue
                esi = earlier.sync_info
                if esi is None or esi.on_wait:
                    continue
                w = extras[-1]
                p = producer_idx(w.id, w.wait_value)
                if p is None:
                    continue
                prod = insts[p]
                psi = prod.sync_info
                same_queue = str(prod.engine) == eng
                blocked_prod = psi is not None and bool(psi.on_wait)
                if p >= eidx and (same_queue or blocked_prod):
                    continue  # placing here could form a wait cycle
                esi.on_wait = [extras.pop()]
            si.on_wait = keep + extras
            assert len(si.on_wait) <= 1, (
                f"could not push waits earlier from {inst.name}")


def _spread_adjacent_waits(nc):
    """Move excess waits from a shield ACT/Memset onto the next zero-wait
    instructions of the same engine queue. Queue order keeps the wait ahead
    of every later instruction on that engine, which is exactly the WAR
    ordering the wait protects."""
    for block in nc.m.functions[0].blocks:
        insts = list(block.instructions)
        for idx, inst in enumerate(insts):
            si = inst.sync_info
            if (type(inst).__name__ not in ("InstActivation", "InstMemset")
                    or not si or len(si.on_wait) <= 1):
                continue
            waits = list(si.on_wait)
            extras = waits[1:]
            eng = str(inst.engine)
            for later in insts[idx + 1:idx + 12]:
                if not extras:
                    break
                if str(later.engine) != eng:
                    continue
                lsi = later.sync_info
                if lsi is None or lsi.on_wait:
                    break
                lsi.on_wait = [extras.pop(0)]
            si.on_wait = waits[:1]
            assert not extras, (
                f"could not spread {len(extras)} waits from {inst.name}")


def _split_tail_drain_waits(nc):
    """walrus allows one sync-wait per instruction; the kernel-tail drain can
    carry one wait per outstanding DMA lane. Redistribute the extras onto
    later same-engine drains whose waits are all trivial (value 0)."""
    for block in nc.m.functions[0].blocks:
        insts = list(block.instructions)
        for idx, inst in enumerate(insts):
            si = inst.sync_info
            if type(inst).__name__ != "InstDrain" or not si or len(si.on_wait) <= 1:
                continue
            waits = list(si.on_wait)
            extras = waits[1:]
            si.on_wait = waits[:1]
            eng = str(inst.engine)
            # prefer same-engine drains, then any pre-barrier drain: every
            # engine rendezvouses at the exit barrier, so a DMA-completion
            # wait on any drain still precedes kernel exit.
            for same_engine in (True, False):
                for later in insts[idx + 1:]:
                    if not extras:
                        break
                    lsi = later.sync_info
                    if (type(later).__name__ == "InstDrain"
                            and (str(later.engine) == eng) == same_engine
                            and lsi is not None
                            and all(w.wait_value == 0 for w in lsi.on_wait)):
                        lsi.on_wait = [extras.pop(0)]
            assert not extras, (
                f"could not redistribute {len(extras)} drain waits on {inst.name}")


def _build_nc(strip=True):
    nc = bass.Bass()
    xs = nc.dram_tensor("xs", [CIN, SHARD_D, PLANE], _MMDT,
                        kind="ExternalInput")
    xsh = nc.dram_tensor("xsh", [CIN, NDW, PLANE_H], _MMDT,
                         kind="ExternalInput")
    wb = nc.dram_tensor("wb", [128, 9 * 96], _MMDT,
                        kind="ExternalInput")
    bs = nc.dram_tensor("bs", [96, 1], mybir.dt.float32, kind="ExternalInput")
    out = nc.dram_tensor("out", [COUT, DSUB * NFULL, S * S], _IODT,
                         kind="ExternalOutput")
    outh = nc.dram_tensor("outh", [MVH, SH * S], _IODT, kind="ExternalOutput")

    with ExitStack() as ctx:
        tc = ctx.enter_context(tile.TileContext(nc))
        consts = ctx.enter_context(tc.tile_pool(name="consts", bufs=1))
        xpool = ctx.enter_context(tc.tile_pool(name="xpool", bufs=NFULL))
        xhpool = ctx.enter_context(tc.tile_pool(name="xhpool", bufs=1))
        opool = ctx.enter_context(tc.tile_pool(name="opool", bufs=2))
        ohpool = ctx.enter_context(tc.tile_pool(name="ohpool", bufs=1))
        pspool = ctx.enter_context(tc.tile_pool(name="pspool", bufs=7, space="PSUM"))

        shield = ctx.enter_context(tc.tile_pool(name="shield", bufs=1, space="PSUM"))
        sps = shield.tile([2, 512], mybir.dt.float32)
        ssb = consts.tile([1, 8], mybir.dt.float32)

        # PE warm-up: dependency-free matmuls keep the PE busy from right
        # after the preamble, so the HAM clock-gate ramps toward full rate
        # while the first x piece is still in flight. The memset runs on the
        # gpsimd engine, whose preamble finishes first. Garbage results land
        # in a scratch psum bank nobody reads.
        warm = consts.tile([128, 512], _MMDT)
        nc.gpsimd.memset(warm, 0.0)
        nc.vector.memset(ssb, 0.0)
        for _ in range(8):
            nc.tensor.matmul(sps[0:2, 0:512], warm[:, 0:2], warm[:, 0:512],
                             start=True, stop=True)

        # x loads on the sync HWDGE ring, in consumption order; block 0 leads
        # with a 9-row piece so chunk 0 can start as early as possible.
        xts = []
        for blk in range(NFULL):
            xt = xpool.tile([128, PLANE + 2 * MARGIN], _MMDT, tag="xt")
            # chunk-0 tap-0 reads one col left of the loaded plane (into a
            # discarded psum column); zero it so the read is defined
            nc.vector.memset(xt[:, MARGIN - 2:MARGIN], 0.0)
            dr0 = DSUB * blk
            # src iterates (cin, dw, plane) -> partition p = cin*8+dw
            if blk == 0:
                nc.sync.dma_start(out=xt[:, MARGIN:MARGIN + P0],
                                  in_=xs[:, dr0:dr0 + NDW, 0:P0])
                nc.sync.dma_start(out=xt[:, MARGIN + P0:MARGIN + XSPLIT],
                                  in_=xs[:, dr0:dr0 + NDW, P0:XSPLIT])
            else:
                nc.sync.dma_start(out=xt[:, MARGIN:MARGIN + XSPLIT],
                                  in_=xs[:, dr0:dr0 + NDW, 0:XSPLIT])
            nc.sync.dma_start(out=xt[:, MARGIN + XSPLIT:MARGIN + PLANE],
                              in_=xs[:, dr0:dr0 + NDW, XSPLIT:PLANE])
            xts.append(xt)
        xth = xhpool.tile([128, PLANE_H + 2 * MARGIN], _MMDT)
        nc.vector.memset(xth[:, MARGIN - 2:MARGIN], 0.0)
        nc.sync.dma_start(out=xth[:, MARGIN:MARGIN + PLANE_H],
                          in_=xsh[:, :, :])

        # weights + bias on the scalar (ACT) ring, in parallel with x loads;
        # the opening matmul only needs wtile[:, 0:96].
        wtile = consts.tile([128, 9 * 96], _MMDT)
        nc.scalar.dma_start(out=wtile[:, 0:96], in_=wb[:, 0:96])
        nc.scalar.dma_start(out=wtile[:, 96:], in_=wb[:, 96:])
        btile = consts.tile([96, 1], mybir.dt.float32)
        nc.scalar.dma_start(out=btile, in_=bs[:, :])

        # walrus allows only one sync-wait on a Matmult; absorb each DMA's
        # completion wait with a dummy 2x2 PE / 1-elem ACT op reading the tile.
        nc.tensor.matmul(sps[0:2, 0:2], wtile[0:2, 0:2], wtile[0:2, 0:2],
                         start=True, stop=True)
        nc.scalar.activation(ssb[0:1, 0:1], btile[0:1, 0:1],
                             mybir.ActivationFunctionType.Copy)

        def emit_block(blk, xt, prev_ob, half):
            """One d-block: 10 psum chunks x 9 taps, extraction, stores."""
            wp = WPH if half else WP
            sw = SH if half else S
            obw = OBWH if half else OBW
            mv_io = MVH if half else 96
            # absorb the xt DMA waits (one per load piece) on the PE engine.
            # The rhs operand anchors each shield to the previous block's
            # extraction progress — without the anchor the scheduler hoists
            # every shield to the front of the PE queue, where they stall it
            # on loads whole blocks ahead of use. Piece A is needed at chunk
            # 0 (anchor: prev ext c5), piece B at chunk 3 (anchor: prev ext
            # c8; for block 0 it is emitted after chunk 0's extraction).
            rhs_a = (xt[0:2, MARGIN:MARGIN + 2] if prev_ob is None
                     else prev_ob[0:2, 2241:2243])
            nc.tensor.matmul(sps[0:2, 2:4], xt[0:2, MARGIN:MARGIN + 2],
                             rhs_a, start=True, stop=True)
            if not half and prev_ob is not None:
                nc.tensor.matmul(
                    sps[0:2, 4:6], xt[0:2, MARGIN + XSPLIT:MARGIN + XSPLIT + 2],
                    prev_ob[0:2, 3585:3587], start=True, stop=True)
            if half:
                ob = ohpool.tile([MVH, SH * S], _IODT)
            else:
                ob = opool.tile([96, S * S], _IODT, tag="ob")
                # absorb the ob-slot-release (out DMA) waits on the ACT
                # engine (one per store of the slot's previous user)
                nc.scalar.activation(ob[0:1, 0:1], ssb[0:1, 0:1],
                                     mybir.ActivationFunctionType.Copy)
                nc.scalar.activation(ob[0:1, 2048:2049], ssb[0:1, 1:2],
                                     mybir.ActivationFunctionType.Copy)
                nc.scalar.activation(ob[0:1, 4032:4033], ssb[0:1, 2:3],
                                     mybir.ActivationFunctionType.Copy)
            for c in range(10):
                rc = CROWS if c < 9 else 1
                ncols = wp * rc
                ps = pspool.tile([96, 512], mybir.dt.float32, tag="ps")
                # absorb the psum-slot-release waits on DVE so the chunk's
                # first matmul carries at most one wait
                nc.vector.memset(ps[0:1, 0:1], 0.0)
                base = MARGIN + wp * (1 + CROWS * c)
                for t in range(9):
                    kh, kw = divmod(t, 3)
                    off = base + (kh - 1) * wp + (kw - 1)
                    nc.tensor.matmul(
                        ps[:96, :ncols],
                        wtile[:, t * 96:(t + 1) * 96],
                        xt[:, off:off + ncols],
                        start=(t == 0),
                        stop=(t == 8),
                    )
                if rc > 1:
                    src = ps[:mv_io, 1:1 + rc * wp].rearrange(
                        "p (r s) -> p r s", r=rc)[:, :, 0:sw]
                    dst = ob[:mv_io, obw * c:obw * c + rc * sw].rearrange(
                        "p (r s) -> p r s", r=rc)
                else:
                    src = ps[:mv_io, 1:1 + sw]
                    dst = ob[:mv_io, obw * c:obw * c + sw]
                nc.scalar.activation(
                    out=dst, in_=src,
                    func=mybir.ActivationFunctionType.Identity,
                    bias=btile[:mv_io, :],
                )
                if blk == 0 and c == 0:
                    # block 0's piece-B shield, anchored to chunk 0's
                    # extraction so it cannot stall the PE queue at t=0
                    nc.tensor.matmul(
                        sps[0:2, 4:6],
                        xt[0:2, MARGIN + XSPLIT:MARGIN + XSPLIT + 2],
                        ob[0:2, 0:2], start=True, stop=True)
                if half:
                    # the closing half block stores on the sync HWDGE ring:
                    # the gpsimd SWDGE queue is still draining the big full-
                    # block stores, and sync has been idle since the loads.
                    if c == 4:
                        nc.sync.dma_start(out=outh[:, 0:5 * OBWH],
                                          in_=ob[:MVH, 0:5 * OBWH])
                    elif c == 7:
                        nc.sync.dma_start(out=outh[:, 5 * OBWH:8 * OBWH],
                                          in_=ob[:MVH, 5 * OBWH:8 * OBWH])
                    elif c == 9:
                        nc.sync.dma_start(out=outh[:, 8 * OBWH:],
                                          in_=ob[:MVH, 8 * OBWH:])
                elif c in (4, 8):
                    # store finished columns as soon as their chunks extract;
                    # dest iterates (ds, co, cols) = partition ds*16+co.
                    # SWDGE issue from the idle GpSimd engine: stores never
                    # touch the HWDGE load lanes, so load-consumer waits (PE
                    # shields) can never alias a store's completion count.
                    lo, hi = (0, 2048) if c == 4 else (2048, 4032)
                    dr0 = DSUB * blk
                    osl = out[:, dr0:dr0 + DSUB, lo:hi]
                    oap = bass.AP(tensor=osl.tensor, offset=osl.offset,
                                  ap=[osl.ap[1], osl.ap[0], osl.ap[2]])
                    nc.gpsimd.dma_start(out=oap, in_=ob[:96, lo:hi])
            if not half:
                dr0 = DSUB * blk
                osl = out[:, dr0:dr0 + DSUB, 4032:4096]
                oap = bass.AP(tensor=osl.tensor, offset=osl.offset,
                              ap=[osl.ap[1], osl.ap[0], osl.ap[2]])
                nc.gpsimd.dma_start(out=oap, in_=ob[:96, 4032:4096])
            return ob

        prev_ob = None
        for blk in range(NFULL):
            prev_ob = emit_block(blk, xts[blk], prev_ob, half=False)
        emit_block(NFULL, xth, prev_ob, half=True)
    if strip:
        # walrus wait-slot legalization; provably equivalent on in-order
        # hardware, but drops same-engine waits CoreSim's detector expects,
        # so the sim path (strip=False) validates the raw Tile program.
        _strip_implied_waits(nc)
        _spread_adjacent_waits(nc)
        _pair_waits_to_ldw(nc)
        _push_waits_earlier(nc)
        _split_tail_drain_waits(nc)
    return nc


def _host_prep(x, weight, bias):
    x = np.ascontiguousarray(x, dtype=np.float32)
    weight = np.ascontiguousarray(weight, dtype=np.float32)
    bias = np.ascontiguousarray(bias, dtype=np.float32)
    xb = x.astype(_NPDT)

    # zero-padded volume, plane rows packed at stride WP=65 (shared pad
    # column trick): col 0 of each row is zero and doubles as the previous
    # row's right pad. Padded plane index = d+1; 66 planes cover d -1..64.
    xp = np.zeros((B, CIN, HP, PLANE), dtype=_NPDT)
    xpv = xp[:, :, :, :HP * WP].reshape(B, CIN, HP, HP, WP)
    xpv[:, :, 1:S + 1, 1:S + 1, 1:S + 1] = xb

    # half-block volume: d planes 29..36, one w-half per core, rows packed
    # at stride WPH=34. Left half: [0, w0..w32]; right half: [w31..w63, 0].
    xh = np.zeros((2, B, CIN, NDW, PLANE_H), dtype=_NPDT)
    xhv = xh[:, :, :, :, :HP * WPH].reshape(2, B, CIN, NDW, HP, WPH)
    xhv[0, :, :, :, 1:S + 1, 1:34] = xb[:, :, 29:37, :, 0:33]
    xhv[1, :, :, :, 1:S + 1, 0:33] = xb[:, :, 29:37, :, 31:64]

    # banded weights: wbd[(cin,dw), t=(kh,kw), (ds,co)]
    wbd = np.zeros((CIN, NDW, 9, 96), dtype=_NPDT)
    wt = weight.astype(_NPDT).transpose(1, 0, 2, 3, 4).reshape(CIN, COUT, 3, 9)
    for ds in range(DSUB):
        for kd in range(3):
            wbd[:, ds + kd, :, ds * 16:(ds + 1) * 16] = wt[:, :, kd, :].transpose(0, 2, 1)
    wbd = np.ascontiguousarray(wbd.reshape(128, 9 * 96))

    bias96 = np.ascontiguousarray(np.tile(bias, DSUB)[:, None])

    in_maps = []
    for core in range(8):
        b, h = divmod(core, 2)
        d0 = 0 if h == 0 else 34
        xsh_ = np.ascontiguousarray(
            xp[b, :, d0:d0 + SHARD_D].reshape(CIN, SHARD_D, PLANE))
        in_maps.append({"xs": xsh_, "xsh": np.ascontiguousarray(xh[h, b]),
                        "wb": wbd, "bs": bias96})
    return in_maps


def kernel(x, weight, bias):
    global _nc_cache, LAST_RESULT
    if _nc_cache is None:
        _nc_cache = _build_nc()
    nc = _nc_cache

    in_maps = _host_prep(x, weight, bias)
    trace = bool(int(os.environ.get("KERNEL_TRACE", "0")))
    res = run_bass_kernel_spmd(nc, in_maps, core_ids=list(range(8)), trace=trace)
    LAST_RESULT = res

    out = np.empty((B, COUT, S, S, S), dtype=np.float32)
    for core in range(8):
        b, h = divmod(core, 2)
        o = res.results[core]["out"].astype(np.float32).reshape(
            COUT, DSUB * NFULL, S, S)
        oh = res.results[core]["outh"].astype(np.float32).reshape(
            4, COUT, S, SH)
        if h == 0:
            out[b, :, 0:30] = o
            out[b, :, 30:34, :, 0:32] = oh.transpose(1, 0, 2, 3)
        else:
            out[b, :, 34:64] = o
            out[b, :, 30:34, :, 32:64] = oh.transpose(1, 0, 2, 3)
    return out


# revision 12
# speedup vs baseline: 1.0038x; 1.0038x over previous
"""Trainium2 Bass kernel for 3D conv: x[4,16,64,64,64] * w[16,16,3,3,3] + bias, pad=1.

Strategy (8 cores): per batch, the 64 output d-planes are split between a core
pair as 5 full-width d-blocks each (A: d[0,30), B: d[34,64)) plus one shared
half-width block for d[30,34) (A computes w[0,32), B computes w[32,64)), so
every core streams 5 full planes + 1 half plane instead of 6 full planes.
Per core, a "banded weight" matmul decomposition:
  - contraction K = (cin=16) x (d-window=8) = 128 partitions
  - output    M = (d_sub=6 outputs) x (cout=16) = 96 partitions
  - the 9 (kh,kw) taps are free-dim shifts over a zero-padded (h,w) plane
  - lhsT[(cin,dw), (ds,co)] = W[co,cin,dw-ds,kh,kw] for 0<=dw-ds<=2 (banded)
Matmuls in bf16 (fp32 psum accumulation); Ldweights fully hidden at steady
state. Bias is fused into the PSUM->SBUF extraction copy on the scalar engine.
DMA: x shards stream on the sync HWDGE ring (block-0's first rows lead),
weights/bias load on the scalar ring in parallel; full-block outputs store via
gpsimd SWDGE; the final half-block stores ride the then-idle sync ring.
"""

import os
from contextlib import ExitStack

import ml_dtypes
import numpy as np

import concourse.bass as bass
import concourse.mybir as mybir
import concourse.tile as tile
from concourse.bass_utils import run_bass_kernel_spmd

_MMDT = mybir.dt.bfloat16
_IODT = mybir.dt.bfloat16
_NPDT = ml_dtypes.bfloat16

B, CIN, COUT, S = 4, 16, 16, 64
HP = S + 2                 # padded rows: 66 (zero rows 0 and 65)
WP = S + 1                 # row stride 65: each row's right pad IS the next
                           # row's left pad (col 0 of every row is zero)
PLANE = HP * WP + 1        # 4291; +1 so the (kh,kw)=(+1,+1) corner read of
                           # the last valid element lands on a host zero
MARGIN = 68                # free-dim margin so shifted reads stay in-bounds
DSUB = 6                   # d outputs per full block
NDW = 8                    # d-window planes (DSUB + 2 halo)
NFULL = 5                  # full-width d-blocks per core (d outputs 0..29)
SHARD_D = 32               # padded d planes per shard (windows span 5 blocks)
CROWS = 7                  # padded h-rows per psum chunk (7*66=462 <= 512)
OBW = CROWS * S            # 448 output cols per full chunk
P0 = 9 * WP                # 585: block-0 lead piece (chunk 0 reads rows [0,9))
XSPLIT = 24 * WP           # piece-A/piece-B split: rows [0,24) / [24,66)

# half-width block (d outputs 30..33, one w-half per core)
SH = 32                    # output cols per row in the half block
WPH = SH + 2               # half-plane row stride: [halo/pad, 32 outputs, halo]
PLANE_H = HP * WPH + 1     # 2245
OBWH = CROWS * SH          # 224
MVH = 64                   # stored partitions: (ds=4 used of 6) x 16

_nc_cache = None
LAST_RESULT = None         # BassKernelResults of the most recent run (for test.py)

# head/tail micro-optimizations, independently toggleable for bisection
GPSIMD_WARM_MEMSET = bool(int(os.environ.get("K_GPSIMD_MEMSET", "0")))
SCALAR_RING_LOADS = bool(int(os.environ.get("K_SCALAR_LOADS", "0")))
SYNC_RING_STORES = bool(int(os.environ.get("K_SYNC_STORES", "0")))


def _strip_implied_waits(nc):
    """Remove semaphore waits that are transitively implied by another wait on
    the same instruction.

    Tile's add_semaphores emits the full non-transitive closure, so a matmul
    whose psum slot was last touched by (PE writes -> ACT read -> DVE memset)
    carries three waits — but walrus only supports a single sync-wait on a
    Matmult (fp32r matmuls are self-loading, and the wait rides the LDW
    struct). A wait (s >= v) is provably redundant if another wait on the
    same instruction targets a producer whose completion already implies
    (s >= v). We replay the scheduled instruction stream with vector clocks
    to compute each semaphore event's implied clock, then drop implied waits.

    In-order completion is assumed per compute-engine queue but NOT for DMA
    instructions (SDMA engines complete packets out of order), so DMA clocks
    only carry their own waits + update.
    """
    sem_count = {}
    sem_events = {}        # sem id -> list of (value_after, clock dict)
    engine_clock = {}
    engine_self = {}       # engine -> {sem id -> updates issued by that engine}

    def join(a, b):
        for k, v in b.items():
            if a.get(k, -1) < v:
                a[k] = v

    def snapshot(sid, val):
        for value_after, clk in sem_events.get(sid, ()):
            if value_after >= val:
                return clk
        return None

    for block in nc.m.functions[0].blocks:
        for inst in block.instructions:
            si = inst.sync_info
            if si is None:
                continue
            eng0 = str(inst.engine)
            is_dma0 = type(inst).__name__ in ("InstDMACopy", "InstDMATranspose")
            is_serial = (not is_dma0 and type(inst).__name__ not in
                         ("InstMatmult", "InstDrain", "InstEventSemaphore"))
            waits = list(si.on_wait)
            if is_serial and len(waits) > 1:
                # serial engines execute in order: a wait on the engine's own
                # completion semaphore for a value its predecessors already
                # produce is a no-op.
                own = engine_self.get(eng0, {})
                kept = [w for w in waits
                        if not (w.wait_mode == "sem-ge-imm"
                                and own.get(w.id, 0) >= w.wait_value)]
                if len(kept) < len(waits):
                    si.on_wait = kept
                    waits = kept
            snaps = []
            for w in waits:
                snaps.append(snapshot(w.id, w.wait_value)
                             if w.wait_mode == "sem-ge-imm" else None)
            if len(waits) > 1:
                keep = []
                for i, w in enumerate(waits):
                    if w.wait_mode != "sem-ge-imm":
                        keep.append(w)
                        continue
                    implied = False
                    for j, other in enumerate(waits):
                        if i == j or snaps[j] is None:
                            continue
                        if snaps[j].get(w.id, -1) >= w.wait_value:
                            implied = True
                            break
                    if not implied:
                        keep.append(w)
                if len(keep) < len(waits):
                    si.on_wait = keep
                    waits = keep
                    snaps = [snapshot(w.id, w.wait_value)
                             if w.wait_mode == "sem-ge-imm" else None
                             for w in waits]

            clk = {}
            for s in snaps:
                if s is not None:
                    join(clk, s)
            eng = str(inst.engine)
            is_dma = type(inst).__name__ in ("InstDMACopy", "InstDMATranspose")
            if not is_dma and eng in engine_clock:
                join(clk, engine_clock[eng])
            for u in si.on_update:
                if u.update_mode == "sem-add-imm":
                    sem_count[u.id] = sem_count.get(u.id, 0) + u.update_value
                elif u.update_mode == "sem-inc":
                    sem_count[u.id] = sem_count.get(u.id, 0) + 1
                else:
                    continue
                clk[u.id] = max(clk.get(u.id, 0), sem_count[u.id])
                sem_events.setdefault(u.id, []).append((sem_count[u.id], clk))
                if not is_dma:
                    es = engine_self.setdefault(eng, {})
                    es[u.id] = sem_count[u.id]
            if not is_dma:
                engine_clock[eng] = clk


def _pair_waits_to_ldw(nc):
    """A bf16 matmul is an Ldweights+Matmult pair; each half carries one
    sync-wait slot. If the Matmult holds two waits and its paired Ldweights
    holds none, move one over (gating the pair earlier is safe)."""
    import bass_rust
    for block in nc.m.functions[0].blocks:
        insts = list(block.instructions)
        for idx, inst in enumerate(insts):
            si = inst.sync_info
            if (type(inst).__name__ != "InstMatmult" or not si
                    or len(si.on_wait) <= 1 or idx == 0):
                continue
            prev = insts[idx - 1]
            if type(prev).__name__ != "InstLdweights":
                continue
            psi = prev.sync_info
            if psi is not None and psi.on_wait:
                continue
            waits = list(si.on_wait)
            moved = waits.pop()
            if psi is None:
                prev.sync_info = bass_rust.SyncInfo(on_wait=[moved],
                                                    on_update=[])
            else:
                psi.on_wait = [moved]
            si.on_wait = waits


def _push_waits_earlier(nc):
    """For a DMACopy still carrying >1 waits (e.g. a DMAHW lane-ordering wait
    plus a data wait), move the extras onto an earlier zero-wait instruction
    of the same engine queue. Satisfying a wait earlier in the queue is
    strictly more conservative — provided the instruction that produces the
    awaited semaphore value is not itself issued later on that queue (which
    would deadlock). Producers are located with a semaphore replay."""
    for block in nc.m.functions[0].blocks:
        insts = list(block.instructions)
        # replay semaphore counts to locate each (sem, value)'s producer idx
        counts = {}
        events = {}  # sem id -> list of (value_after, idx)
        for idx, inst in enumerate(insts):
            si = inst.sync_info
            if not si:
                continue
            for u in si.on_update:
                inc = u.update_value if u.update_mode == "sem-add-imm" else (
                    1 if u.update_mode == "sem-inc" else 0)
                if not inc:
                    continue
                counts[u.id] = counts.get(u.id, 0) + inc
                events.setdefault(u.id, []).append((counts[u.id], idx))

        def producer_idx(sid, val):
            for value_after, idx in events.get(sid, ()):
                if value_after >= val:
                    return idx
            return None

        for idx, inst in enumerate(insts):
            si = inst.sync_info
            if (type(inst).__name__ != "InstDMACopy"
                    or not si or len(si.on_wait) <= 1):
                continue
            waits = list(si.on_wait)
            keep = [w for w in waits if not w.ant_name.startswith("DMAHW")]
            extras = [w for w in waits if w.ant_name.startswith("DMAHW")]
            if len(keep) + min(1, len(extras)) <= 1:
                continue
            eng = str(inst.engine)
            for eidx in range(idx - 1, -1, -1):
                if len(keep) + len(extras) <= 1:
                    break
                earlier = insts[eidx]
                if str(earlier.engine) != eng:
                    continue
                esi = earlier.sync_info
                if esi is None or esi.on_wait:
                    continue
                w = extras[-1]
                p = producer_idx(w.id, w.wait_value)
                if p is None:
                    continue
                prod = insts[p]
                psi = prod.sync_info
                same_queue = str(prod.engine) == eng
                blocked_prod = psi is not None and bool(psi.on_wait)
                if p >= eidx and (same_queue or blocked_prod):
                    continue  # placing here could form a wait cycle
                esi.on_wait = [extras.pop()]
            si.on_wait = keep + extras
            assert len(si.on_wait) <= 1, (
                f"could not push waits earlier from {inst.name}")


def _spread_adjacent_waits(nc):
    """Move excess waits from a shield ACT/Memset onto the next zero-wait
    instructions of the same engine queue. Queue order keeps the wait ahead
    of every later instruction on that engine, which is exactly the WAR
    ordering the wait protects."""
    for block in nc.m.functions[0].blocks:
        insts = list(block.instructions)
        for idx, inst in enumerate(insts):
            si = inst.sync_info
            if (type(inst).__name__ not in ("InstActivation", "InstMemset")
                    or not si or len(si.on_wait) <= 1):
                continue
            waits = list(si.on_wait)
            extras = waits[1:]
            eng = str(inst.engine)
            for later in insts[idx + 1:idx + 12]:
                if not extras:
                    break
                if str(later.engine) != eng:
                    continue
                lsi = later.sync_info
                if lsi is None or lsi.on_wait:
                    break
                lsi.on_wait = [extras.pop(0)]
            si.on_wait = waits[:1]
            assert not extras, (
                f"could not spread {len(extras)} waits from {inst.name}")


def _split_tail_drain_waits(nc):
    """walrus allows one sync-wait per instruction; the kernel-tail drain can
    carry one wait per outstanding DMA lane. Redistribute the extras onto
    later same-engine drains whose waits are all trivial (value 0)."""
    for block in nc.m.functions[0].blocks:
        insts = list(block.instructions)
        for idx, inst in enumerate(insts):
            si = inst.sync_info
            if type(inst).__name__ != "InstDrain" or not si or len(si.on_wait) <= 1:
                continue
            waits = list(si.on_wait)
            extras = waits[1:]
            si.on_wait = waits[:1]
            eng = str(inst.engine)
            # prefer same-engine drains, then any pre-barrier drain: every
            # engine rendezvouses at the exit barrier, so a DMA-completion
            # wait on any drain still precedes kernel exit.
            for same_engine in (True, False):
                for later in insts[idx + 1:]:
                    if not extras:
                        break
                    lsi = later.sync_info
                    if (type(later).__name__ == "InstDrain"
                            and (str(later.engine) == eng) == same_engine
                            and lsi is not None
                            and all(w.wait_value == 0 for w in lsi.on_wait)):
                        lsi.on_wait = [extras.pop(0)]
            assert not extras, (
                f"could not redistribute {len(extras)} drain waits on {inst.name}")


def _build_nc(strip=True):
    nc = bass.Bass()
    xs = nc.dram_tensor("xs", [CIN, SHARD_D, PLANE], _MMDT,
                        kind="ExternalInput")
    xsh = nc.dram_tensor("xsh", [CIN, NDW, PLANE_H], _MMDT,
                         kind="ExternalInput")
    wb = nc.dram_tensor("wb", [128, 9 * 96], _MMDT,
                        kind="ExternalInput")
    bs = nc.dram_tensor("bs", [96, 1], mybir.dt.float32, kind="ExternalInput")
    out = nc.dram_tensor("out", [COUT, DSUB * NFULL, S * S], _IODT,
                         kind="ExternalOutput")
    outh = nc.dram_tensor("outh", [MVH, SH * S], _IODT, kind="ExternalOutput")

    with ExitStack() as ctx:
        tc = ctx.enter_context(tile.TileContext(nc))
        consts = ctx.enter_context(tc.tile_pool(name="consts", bufs=1))
        xpool = ctx.enter_context(tc.tile_pool(name="xpool", bufs=NFULL))
        xhpool = ctx.enter_context(tc.tile_pool(name="xhpool", bufs=1))
        opool = ctx.enter_context(tc.tile_pool(name="opool", bufs=2))
        ohpool = ctx.enter_context(tc.tile_pool(name="ohpool", bufs=1))
        pspool = ctx.enter_context(tc.tile_pool(name="pspool", bufs=7, space="PSUM"))

        shield = ctx.enter_context(tc.tile_pool(name="shield", bufs=1, space="PSUM"))
        sps = shield.tile([2, 512], mybir.dt.float32)
        ssb = consts.tile([1, 8], mybir.dt.float32)

        # PE warm-up: dependency-free matmuls keep the PE busy from right
        # after the preamble, so the HAM clock-gate ramps toward full rate
        # while the first x piece is still in flight. The memset runs on the
        # gpsimd engine, whose preamble finishes first. Garbage results land
        # in a scratch psum bank nobody reads.
        warm = consts.tile([128, 512], _MMDT)
        if GPSIMD_WARM_MEMSET:
            nc.gpsimd.memset(warm, 0.0)
        else:
            nc.vector.memset(warm, 0.0)
        nc.vector.memset(ssb, 0.0)
        for _ in range(8):
            nc.tensor.matmul(sps[0:2, 0:512], warm[:, 0:2], warm[:, 0:512],
                             start=True, stop=True)

        wtile = consts.tile([128, 9 * 96], _MMDT)
        btile = consts.tile([96, 1], mybir.dt.float32)
        if not SCALAR_RING_LOADS:
            # conservative: weights + bias lead the sync ring (baseline order)
            nc.sync.dma_start(out=wtile[:, 0:96], in_=wb[:, 0:96])
            nc.sync.dma_start(out=wtile[:, 96:], in_=wb[:, 96:])
            nc.sync.dma_start(out=btile, in_=bs[:, :])

        # x loads on the sync HWDGE ring, in consumption order; block 0 leads
        # with a 9-row piece so chunk 0 can start as early as possible.
        xts = []
        for blk in range(NFULL):
            xt = xpool.tile([128, PLANE + 2 * MARGIN], _MMDT, tag="xt")
            # chunk-0 tap-0 reads one col left of the loaded plane (into a
            # discarded psum column); zero it so the read is defined
            nc.vector.memset(xt[:, MARGIN - 2:MARGIN], 0.0)
            dr0 = DSUB * blk
            # src iterates (cin, dw, plane) -> partition p = cin*8+dw
            if blk == 0:
                nc.sync.dma_start(out=xt[:, MARGIN:MARGIN + P0],
                                  in_=xs[:, dr0:dr0 + NDW, 0:P0])
                nc.sync.dma_start(out=xt[:, MARGIN + P0:MARGIN + XSPLIT],
                                  in_=xs[:, dr0:dr0 + NDW, P0:XSPLIT])
            else:
                nc.sync.dma_start(out=xt[:, MARGIN:MARGIN + XSPLIT],
                                  in_=xs[:, dr0:dr0 + NDW, 0:XSPLIT])
            nc.sync.dma_start(out=xt[:, MARGIN + XSPLIT:MARGIN + PLANE],
                              in_=xs[:, dr0:dr0 + NDW, XSPLIT:PLANE])
            xts.append(xt)
        xth = xhpool.tile([128, PLANE_H + 2 * MARGIN], _MMDT)
        nc.vector.memset(xth[:, MARGIN - 2:MARGIN], 0.0)
        nc.sync.dma_start(out=xth[:, MARGIN:MARGIN + PLANE_H],
                          in_=xsh[:, :, :])

        if SCALAR_RING_LOADS:
            # weights + bias on the scalar (ACT) ring, in parallel with the
            # x loads; the opening matmul only needs wtile[:, 0:96].
            nc.scalar.dma_start(out=wtile[:, 0:96], in_=wb[:, 0:96])
            nc.scalar.dma_start(out=wtile[:, 96:], in_=wb[:, 96:])
            nc.scalar.dma_start(out=btile, in_=bs[:, :])

        # walrus allows only one sync-wait on a Matmult; absorb each DMA's
        # completion wait with a dummy 2x2 PE / 1-elem ACT op reading the tile.
        nc.tensor.matmul(sps[0:2, 0:2], wtile[0:2, 0:2], wtile[0:2, 0:2],
                         start=True, stop=True)
        nc.scalar.activation(ssb[0:1, 0:1], btile[0:1, 0:1],
                             mybir.ActivationFunctionType.Copy)

        def emit_block(blk, xt, prev_ob, half):
            """One d-block: 10 psum chunks x 9 taps, extraction, stores."""
            wp = WPH if half else WP
            sw = SH if half else S
            obw = OBWH if half else OBW
            mv_io = MVH if half else 96
            # absorb the xt DMA waits (one per load piece) on the PE engine.
            # The rhs operand anchors each shield to the previous block's
            # extraction progress — without the anchor the scheduler hoists
            # every shield to the front of the PE queue, where they stall it
            # on loads whole blocks ahead of use. Piece A is needed at chunk
            # 0 (anchor: prev ext c5), piece B at chunk 3 (anchor: prev ext
            # c8; for block 0 it is emitted after chunk 0's extraction).
            rhs_a = (xt[0:2, MARGIN:MARGIN + 2] if prev_ob is None
                     else prev_ob[0:2, 2241:2243])
            nc.tensor.matmul(sps[0:2, 2:4], xt[0:2, MARGIN:MARGIN + 2],
                             rhs_a, start=True, stop=True)
            if not half and prev_ob is not None:
                nc.tensor.matmul(
                    sps[0:2, 4:6], xt[0:2, MARGIN + XSPLIT:MARGIN + XSPLIT + 2],
                    prev_ob[0:2, 3585:3587], start=True, stop=True)
            if half:
                ob = ohpool.tile([MVH, SH * S], _IODT)
            else:
                ob = opool.tile([96, S * S], _IODT, tag="ob")
                # absorb the ob-slot-release (out DMA) waits on the ACT
                # engine (one per store of the slot's previous user)
                nc.scalar.activation(ob[0:1, 0:1], ssb[0:1, 0:1],
                                     mybir.ActivationFunctionType.Copy)
                nc.scalar.activation(ob[0:1, 2048:2049], ssb[0:1, 1:2],
                                     mybir.ActivationFunctionType.Copy)
                nc.scalar.activation(ob[0:1, 4032:4033], ssb[0:1, 2:3],
                                     mybir.ActivationFunctionType.Copy)
            for c in range(10):
                rc = CROWS if c < 9 else 1
                ncols = wp * rc
                ps = pspool.tile([96, 512], mybir.dt.float32, tag="ps")
                # absorb the psum-slot-release waits on DVE so the chunk's
                # first matmul carries at most one wait
                nc.vector.memset(ps[0:1, 0:1], 0.0)
                base = MARGIN + wp * (1 + CROWS * c)
                for t in range(9):
                    kh, kw = divmod(t, 3)
                    off = base + (kh - 1) * wp + (kw - 1)
                    nc.tensor.matmul(
                        ps[:96, :ncols],
                        wtile[:, t * 96:(t + 1) * 96],
                        xt[:, off:off + ncols],
                        start=(t == 0),
                        stop=(t == 8),
                    )
                if rc > 1:
                    src = ps[:mv_io, 1:1 + rc * wp].rearrange(
                        "p (r s) -> p r s", r=rc)[:, :, 0:sw]
                    dst = ob[:mv_io, obw * c:obw * c + rc * sw].rearrange(
                        "p (r s) -> p r s", r=rc)
                else:
                    src = ps[:mv_io, 1:1 + sw]
                    dst = ob[:mv_io, obw * c:obw * c + sw]
                nc.scalar.activation(
                    out=dst, in_=src,
                    func=mybir.ActivationFunctionType.Identity,
                    bias=btile[:mv_io, :],
                )
                if blk == 0 and c == 0:
                    # block 0's piece-B shield, anchored to chunk 0's
                    # extraction so it cannot stall the PE queue at t=0
                    nc.tensor.matmul(
                        sps[0:2, 4:6],
                        xt[0:2, MARGIN + XSPLIT:MARGIN + XSPLIT + 2],
                        ob[0:2, 0:2], start=True, stop=True)
                if half:
                    # the closing half block's stores: on the sync HWDGE ring
                    # (idle since the loads; gpsimd SWDGE is still draining
                    # the big full-block stores), or conservatively on
                    # gpsimd + a final low-latency scalar-ring sliver.
                    eng1 = nc.sync if SYNC_RING_STORES else nc.gpsimd
                    eng2 = nc.sync if SYNC_RING_STORES else nc.scalar
                    if c == 4:
                        eng1.dma_start(out=outh[:, 0:5 * OBWH],
                                       in_=ob[:MVH, 0:5 * OBWH])
                    elif c == 7:
                        eng1.dma_start(out=outh[:, 5 * OBWH:8 * OBWH],
                                       in_=ob[:MVH, 5 * OBWH:8 * OBWH])
                    elif c == 9:
                        eng2.dma_start(out=outh[:, 8 * OBWH:],
                                       in_=ob[:MVH, 8 * OBWH:])
                elif c in (4, 8):
                    # store finished columns as soon as their chunks extract;
                    # dest iterates (ds, co, cols) = partition ds*16+co.
                    # SWDGE issue from the idle GpSimd engine: stores never
                    # touch the HWDGE load lanes, so load-consumer waits (PE
                    # shields) can never alias a store's completion count.
                    lo, hi = (0, 2048) if c == 4 else (2048, 4032)
                    dr0 = DSUB * blk
                    osl = out[:, dr0:dr0 + DSUB, lo:hi]
                    oap = bass.AP(tensor=osl.tensor, offset=osl.offset,
                                  ap=[osl.ap[1], osl.ap[0], osl.ap[2]])
                    nc.gpsimd.dma_start(out=oap, in_=ob[:96, lo:hi])
            if not half:
                dr0 = DSUB * blk
                osl = out[:, dr0:dr0 + DSUB, 4032:4096]
                oap = bass.AP(tensor=osl.tensor, offset=osl.offset,
                              ap=[osl.ap[1], osl.ap[0], osl.ap[2]])
                nc.gpsimd.dma_start(out=oap, in_=ob[:96, 4032:4096])
            return ob

        prev_ob = None
        for blk in range(NFULL):
            prev_ob = emit_block(blk, xts[blk], prev_ob, half=False)
        emit_block(NFULL, xth, prev_ob, half=True)
    if strip:
        # walrus wait-slot legalization; provably equivalent on in-order
        # hardware, but drops same-engine waits CoreSim's detector expects,
        # so the sim path (strip=False) validates the raw Tile program.
        _strip_implied_waits(nc)
        _spread_adjacent_waits(nc)
        _pair_waits_to_ldw(nc)
        _push_waits_earlier(nc)
        _split_tail_drain_waits(nc)
    return nc


def _host_prep(x, weight, bias):
    x = np.ascontiguousarray(x, dtype=np.float32)
    weight = np.ascontiguousarray(weight, dtype=np.float32)
    bias = np.ascontiguousarray(bias, dtype=np.float32)
    xb = x.astype(_NPDT)

    # zero-padded volume, plane rows packed at stride WP=65 (shared pad
    # column trick): col 0 of each row is zero and doubles as the previous
    # row's right pad. Padded plane index = d+1; 66 planes cover d -1..64.
    xp = np.zeros((B, CIN, HP, PLANE), dtype=_NPDT)
    xpv = xp[:, :, :, :HP * WP].reshape(B, CIN, HP, HP, WP)
    xpv[:, :, 1:S + 1, 1:S + 1, 1:S + 1] = xb

    # half-block volume: d planes 29..36, one w-half per core, rows packed
    # at stride WPH=34. Left half: [0, w0..w32]; right half: [w31..w63, 0].
    xh = np.zeros((2, B, CIN, NDW, PLANE_H), dtype=_NPDT)
    xhv = xh[:, :, :, :, :HP * WPH].reshape(2, B, CIN, NDW, HP, WPH)
    xhv[0, :, :, :, 1:S + 1, 1:34] = xb[:, :, 29:37, :, 0:33]
    xhv[1, :, :, :, 1:S + 1, 0:33] = xb[:, :, 29:37, :, 31:64]

    # banded weights: wbd[(cin,dw), t=(kh,kw), (ds,co)]
    wbd = np.zeros((CIN, NDW, 9, 96), dtype=_NPDT)
    wt = weight.astype(_NPDT).transpose(1, 0, 2, 3, 4).reshape(CIN, COUT, 3, 9)
    for ds in range(DSUB):
        for kd in range(3):
            wbd[:, ds + kd, :, ds * 16:(ds + 1) * 16] = wt[:, :, kd, :].transpose(0, 2, 1)
    wbd = np.ascontiguousarray(wbd.reshape(128, 9 * 96))

    bias96 = np.ascontiguousarray(np.tile(bias, DSUB)[:, None])

    in_maps = []
    for core in range(8):
        b, h = divmod(core, 2)
        d0 = 0 if h == 0 else 34
        xsh_ = np.ascontiguousarray(
            xp[b, :, d0:d0 + SHARD_D].reshape(CIN, SHARD_D, PLANE))
        in_maps.append({"xs": xsh_, "xsh": np.ascontiguousarray(xh[h, b]),
                        "wb": wbd, "bs": bias96})
    return in_maps


def kernel(x, weight, bias):
    global _nc_cache, LAST_RESULT
    if _nc_cache is None:
        _nc_cache = _build_nc()
    nc = _nc_cache

    in_maps = _host_prep(x, weight, bias)
    trace = bool(int(os.environ.get("KERNEL_TRACE", "0")))
    res = run_bass_kernel_spmd(nc, in_maps, core_ids=list(range(8)), trace=trace)
    LAST_RESULT = res

    out = np.empty((B, COUT, S, S, S), dtype=np.float32)
    for core in range(8):
        b, h = divmod(core, 2)
        o = res.results[core]["out"].astype(np.float32).reshape(
            COUT, DSUB * NFULL, S, S)
        oh = res.results[core]["outh"].astype(np.float32).reshape(
            4, COUT, S, SH)
        if h == 0:
            out[b, :, 0:30] = o
            out[b, :, 30:34, :, 0:32] = oh.transpose(1, 0, 2, 3)
        else:
            out[b, :, 34:64] = o
            out[b, :, 30:34, :, 32:64] = oh.transpose(1, 0, 2, 3)
    return out


# revision 14
# speedup vs baseline: 1.0215x; 1.0176x over previous
"""Original baseline kernel (121us) reconstructed for device-health A/B tests.
Reuses the wait-legalization passes from kernel.py (identical code)."""

import os
from contextlib import ExitStack

import ml_dtypes
import numpy as np

import concourse.bass as bass
import concourse.mybir as mybir
import concourse.tile as tile
from concourse.bass_utils import run_bass_kernel_spmd

def _strip_implied_waits(nc):
    """Remove semaphore waits that are transitively implied by another wait on
    the same instruction.

    Tile's add_semaphores emits the full non-transitive closure, so a matmul
    whose psum slot was last touched by (PE writes -> ACT read -> DVE memset)
    carries three waits — but walrus only supports a single sync-wait on a
    Matmult (fp32r matmuls are self-loading, and the wait rides the LDW
    struct). A wait (s >= v) is provably redundant if another wait on the
    same instruction targets a producer whose completion already implies
    (s >= v). We replay the scheduled instruction stream with vector clocks
    to compute each semaphore event's implied clock, then drop implied waits.

    In-order completion is assumed per compute-engine queue but NOT for DMA
    instructions (SDMA engines complete packets out of order), so DMA clocks
    only carry their own waits + update.
    """
    sem_count = {}
    sem_events = {}        # sem id -> list of (value_after, clock dict)
    engine_clock = {}
    engine_self = {}       # engine -> {sem id -> updates issued by that engine}

    def join(a, b):
        for k, v in b.items():
            if a.get(k, -1) < v:
                a[k] = v

    def snapshot(sid, val):
        for value_after, clk in sem_events.get(sid, ()):
            if value_after >= val:
                return clk
        return None

    for block in nc.m.functions[0].blocks:
        for inst in block.instructions:
            si = inst.sync_info
            if si is None:
                continue
            eng0 = str(inst.engine)
            is_dma0 = type(inst).__name__ in ("InstDMACopy", "InstDMATranspose")
            is_serial = (not is_dma0 and type(inst).__name__ not in
                         ("InstMatmult", "InstDrain", "InstEventSemaphore"))
            waits = list(si.on_wait)
            if is_serial and len(waits) > 1:
                # serial engines execute in order: a wait on the engine's own
                # completion semaphore for a value its predecessors already
                # produce is a no-op.
                own = engine_self.get(eng0, {})
                kept = [w for w in waits
                        if not (w.wait_mode == "sem-ge-imm"
                                and own.get(w.id, 0) >= w.wait_value)]
                if len(kept) < len(waits):
                    si.on_wait = kept
                    waits = kept
            snaps = []
            for w in waits:
                snaps.append(snapshot(w.id, w.wait_value)
                             if w.wait_mode == "sem-ge-imm" else None)
            if len(waits) > 1:
                keep = []
                for i, w in enumerate(waits):
                    if w.wait_mode != "sem-ge-imm":
                        keep.append(w)
                        continue
                    implied = False
                    for j, other in enumerate(waits):
                        if i == j or snaps[j] is None:
                            continue
                        if snaps[j].get(w.id, -1) >= w.wait_value:
                            implied = True
                            break
                    if not implied:
                        keep.append(w)
                if len(keep) < len(waits):
                    si.on_wait = keep
                    waits = keep
                    snaps = [snapshot(w.id, w.wait_value)
                             if w.wait_mode == "sem-ge-imm" else None
                             for w in waits]

            clk = {}
            for s in snaps:
                if s is not None:
                    join(clk, s)
            eng = str(inst.engine)
            is_dma = type(inst).__name__ in ("InstDMACopy", "InstDMATranspose")
            if not is_dma and eng in engine_clock:
                join(clk, engine_clock[eng])
            for u in si.on_update:
                if u.update_mode == "sem-add-imm":
                    sem_count[u.id] = sem_count.get(u.id, 0) + u.update_value
                elif u.update_mode == "sem-inc":
                    sem_count[u.id] = sem_count.get(u.id, 0) + 1
                else:
                    continue
                clk[u.id] = max(clk.get(u.id, 0), sem_count[u.id])
                sem_events.setdefault(u.id, []).append((sem_count[u.id], clk))
                if not is_dma:
                    es = engine_self.setdefault(eng, {})
                    es[u.id] = sem_count[u.id]
            if not is_dma:
                engine_clock[eng] = clk


def _pair_waits_to_ldw(nc):
    """A bf16 matmul is an Ldweights+Matmult pair; each half carries one
    sync-wait slot. If the Matmult holds two waits and its paired Ldweights
    holds none, move one over (gating the pair earlier is safe)."""
    import bass_rust
    for block in nc.m.functions[0].blocks:
        insts = list(block.instructions)
        for idx, inst in enumerate(insts):
            si = inst.sync_info
            if (type(inst).__name__ != "InstMatmult" or not si
                    or len(si.on_wait) <= 1 or idx == 0):
                continue
            prev = insts[idx - 1]
            if type(prev).__name__ != "InstLdweights":
                continue
            psi = prev.sync_info
            if psi is not None and psi.on_wait:
                continue
            waits = list(si.on_wait)
            moved = waits.pop()
            if psi is None:
                prev.sync_info = bass_rust.SyncInfo(on_wait=[moved],
                                                    on_update=[])
            else:
                psi.on_wait = [moved]
            si.on_wait = waits


def _push_waits_earlier(nc):
    """For a DMACopy still carrying >1 waits (e.g. a DMAHW lane-ordering wait
    plus a data wait), move the extras onto an earlier zero-wait instruction
    of the same engine queue. Satisfying a wait earlier in the queue is
    strictly more conservative — provided the instruction that produces the
    awaited semaphore value is not itself issued later on that queue (which
    would deadlock). Producers are located with a semaphore replay."""
    for block in nc.m.functions[0].blocks:
        insts = list(block.instructions)
        # replay semaphore counts to locate each (sem, value)'s producer idx
        counts = {}
        events = {}  # sem id -> list of (value_after, idx)
        for idx, inst in enumerate(insts):
            si = inst.sync_info
            if not si:
                continue
            for u in si.on_update:
                inc = u.update_value if u.update_mode == "sem-add-imm" else (
                    1 if u.update_mode == "sem-inc" else 0)
                if not inc:
                    continue
                counts[u.id] = counts.get(u.id, 0) + inc
                events.setdefault(u.id, []).append((counts[u.id], idx))

        def producer_idx(sid, val):
            for value_after, idx in events.get(sid, ()):
                if value_after >= val:
                    return idx
            return None

        for idx, inst in enumerate(insts):
            si = inst.sync_info
            if (type(inst).__name__ != "InstDMACopy"
                    or not si or len(si.on_wait) <= 1):
                continue
            waits = list(si.on_wait)
            keep = [w for w in waits if not w.ant_name.startswith("DMAHW")]
            extras = [w for w in waits if w.ant_name.startswith("DMAHW")]
            if len(keep) + min(1, len(extras)) <= 1:
                continue
            eng = str(inst.engine)
            for eidx in range(idx - 1, -1, -1):
                if len(keep) + len(extras) <= 1:
                    break
                earlier = insts[eidx]
                if str(earlier.engine) != eng:
                    continue
                esi = earlier.sync_info
                if esi is None or esi.on_wait:
                    continue
                w = extras[-1]
                p = producer_idx(w.id, w.wait_value)
                if p is None:
                    continue
                prod = insts[p]
                psi = prod.sync_info
                same_queue = str(prod.engine) == eng
                blocked_prod = psi is not None and bool(psi.on_wait)
                if p >= eidx and (same_queue or blocked_prod):
                    continue  # placing here could form a wait cycle
                esi.on_wait = [extras.pop()]
            si.on_wait = keep + extras
            assert len(si.on_wait) <= 1, (
                f"could not push waits earlier from {inst.name}")


def _spread_adjacent_waits(nc):
    """Move excess waits from a shield ACT/Memset onto the next zero-wait
    instructions of the same engine queue. Queue order keeps the wait ahead
    of every later instruction on that engine, which is exactly the WAR
    ordering the wait protects."""
    for block in nc.m.functions[0].blocks:
        insts = list(block.instructions)
        for idx, inst in enumerate(insts):
            si = inst.sync_info
            if (type(inst).__name__ not in ("InstActivation", "InstMemset")
                    or not si or len(si.on_wait) <= 1):
                continue
            waits = list(si.on_wait)
            extras = waits[1:]
            eng = str(inst.engine)
            for later in insts[idx + 1:idx + 12]:
                if not extras:
                    break
                if str(later.engine) != eng:
                    continue
                lsi = later.sync_info
                if lsi is None or lsi.on_wait:
                    break
                lsi.on_wait = [extras.pop(0)]
            si.on_wait = waits[:1]
            assert not extras, (
                f"could not spread {len(extras)} waits from {inst.name}")


def _split_tail_drain_waits(nc):
    """walrus allows one sync-wait per instruction; the kernel-tail drain can
    carry one wait per outstanding DMA lane. Redistribute the extras onto
    later same-engine drains whose waits are all trivial (value 0)."""
    for block in nc.m.functions[0].blocks:
        insts = list(block.instructions)
        for idx, inst in enumerate(insts):
            si = inst.sync_info
            if type(inst).__name__ != "InstDrain" or not si or len(si.on_wait) <= 1:
                continue
            waits = list(si.on_wait)
            extras = waits[1:]
            si.on_wait = waits[:1]
            eng = str(inst.engine)
            # prefer same-engine drains, then any pre-barrier drain: every
            # engine rendezvouses at the exit barrier, so a DMA-completion
            # wait on any drain still precedes kernel exit.
            for same_engine in (True, False):
                for later in insts[idx + 1:]:
                    if not extras:
                        break
                    lsi = later.sync_info
                    if (type(later).__name__ == "InstDrain"
                            and (str(later.engine) == eng) == same_engine
                            and lsi is not None
                            and all(w.wait_value == 0 for w in lsi.on_wait)):
                        lsi.on_wait = [extras.pop(0)]
            assert not extras, (
                f"could not redistribute {len(extras)} drain waits on {inst.name}")

_MMDT = mybir.dt.bfloat16
_IODT = mybir.dt.bfloat16
_NPDT = ml_dtypes.bfloat16

B, CIN, COUT, S = 4, 16, 16, 64
HP = S + 2
WP = S + 1
PLANE = HP * WP + 1
MARGIN = 68
DSUB = 6
NDW = 8
DHALF = 32
SHARD_D = DHALF + 6
BLOCKS = [(0, 6), (6, 6), (12, 6), (18, 6), (24, 6), (30, 2)]
CROWS = 7
OBW = CROWS * S

_nc_cache = None
LAST_RESULT = None


def _build_nc():
    nc = bass.Bass()
    xs = nc.dram_tensor("xs", [CIN, SHARD_D, PLANE], _MMDT, kind="ExternalInput")
    wb = nc.dram_tensor("wb", [128, 9 * 96], _MMDT, kind="ExternalInput")
    bs = nc.dram_tensor("bs", [96, 1], mybir.dt.float32, kind="ExternalInput")
    out = nc.dram_tensor("out", [COUT, DHALF, S * S], _IODT, kind="ExternalOutput")

    with ExitStack() as ctx:
        tc = ctx.enter_context(tile.TileContext(nc))
        consts = ctx.enter_context(tc.tile_pool(name="consts", bufs=1))
        xpool = ctx.enter_context(tc.tile_pool(name="xpool", bufs=6))
        opool = ctx.enter_context(tc.tile_pool(name="opool", bufs=2))
        pspool = ctx.enter_context(tc.tile_pool(name="pspool", bufs=7, space="PSUM"))

        shield = ctx.enter_context(tc.tile_pool(name="shield", bufs=1, space="PSUM"))
        sps = shield.tile([2, 512], mybir.dt.float32)
        ssb = consts.tile([1, 8], mybir.dt.float32)

        warm = consts.tile([128, 512], _MMDT)
        nc.vector.memset(warm, 0.0)
        for _ in range(10):
            nc.tensor.matmul(sps[0:2, 0:512], warm[:, 0:2], warm[:, 0:512],
                             start=True, stop=True)

        wtile = consts.tile([128, 9 * 96], _MMDT)
        nc.sync.dma_start(out=wtile[:, 0:96], in_=wb[:, 0:96])
        nc.sync.dma_start(out=wtile[:, 96:], in_=wb[:, 96:])
        btile = consts.tile([96, 1], mybir.dt.float32)
        nc.sync.dma_start(out=btile, in_=bs[:, :])
        XSPLIT = 24 * WP
        xts = []
        for blk, (dr0, dsc) in enumerate(BLOCKS):
            xt = xpool.tile([128, PLANE + 2 * MARGIN], _MMDT, tag="xt")
            if blk == 0:
                p0 = 12 * WP
                nc.sync.dma_start(out=xt[:, MARGIN:MARGIN + p0],
                                  in_=xs[:, dr0:dr0 + NDW, 0:p0])
                nc.sync.dma_start(out=xt[:, MARGIN + p0:MARGIN + XSPLIT],
                                  in_=xs[:, dr0:dr0 + NDW, p0:XSPLIT])
            else:
                nc.sync.dma_start(
                    out=xt[:, MARGIN:MARGIN + XSPLIT],
                    in_=xs[:, dr0:dr0 + NDW, 0:XSPLIT],
                )
            nc.sync.dma_start(
                out=xt[:, MARGIN + XSPLIT:MARGIN + PLANE],
                in_=xs[:, dr0:dr0 + NDW, XSPLIT:PLANE],
            )
            xts.append(xt)
        nc.tensor.matmul(sps[0:2, 0:2], wtile[0:2, 0:2], wtile[0:2, 0:2],
                         start=True, stop=True)
        nc.scalar.activation(ssb[0:1, 0:1], btile[0:1, 0:1],
                             mybir.ActivationFunctionType.Copy)

        prev_ob = None
        for blk, (dr0, dsc) in enumerate(BLOCKS):
            mv = 16 * dsc
            xt = xts[blk]
            rhs_a = (xt[0:2, MARGIN:MARGIN + 2] if prev_ob is None
                     else prev_ob[0:2, 2241:2243])
            nc.tensor.matmul(sps[0:2, 2:4], xt[0:2, MARGIN:MARGIN + 2],
                             rhs_a, start=True, stop=True)
            if prev_ob is not None:
                nc.tensor.matmul(
                    sps[0:2, 4:6], xt[0:2, MARGIN + XSPLIT:MARGIN + XSPLIT + 2],
                    prev_ob[0:2, 3585:3587], start=True, stop=True)
            ob = opool.tile([96, S * S], _IODT, tag="ob")
            nc.scalar.activation(ob[0:1, 0:1], ssb[0:1, 0:1],
                                 mybir.ActivationFunctionType.Copy)
            nc.scalar.activation(ob[0:1, 2048:2049], ssb[0:1, 1:2],
                                 mybir.ActivationFunctionType.Copy)
            nc.scalar.activation(ob[0:1, 4032:4033], ssb[0:1, 2:3],
                                 mybir.ActivationFunctionType.Copy)
            for c in range(10):
                rc = CROWS if c < 9 else 1
                ncols = WP * rc
                ps = pspool.tile([96, 512], mybir.dt.float32, tag="ps")
                nc.vector.memset(ps[0:1, 0:1], 0.0)
                base = MARGIN + WP * (1 + CROWS * c)
                for t in range(9):
                    kh, kw = divmod(t, 3)
                    off = base + (kh - 1) * WP + (kw - 1)
                    nc.tensor.matmul(
                        ps[:mv, :ncols],
                        wtile[:, t * 96:t * 96 + mv],
                        xt[:, off:off + ncols],
                        start=(t == 0),
                        stop=(t == 8),
                    )
                if rc > 1:
                    src = ps[:mv, 1:1 + rc * WP].rearrange(
                        "p (r s) -> p r s", r=rc)[:, :, 0:S]
                    dst = ob[:mv, OBW * c:OBW * c + rc * S].rearrange(
                        "p (r s) -> p r s", r=rc)
                else:
                    src = ps[:mv, 1:1 + S]
                    dst = ob[:mv, OBW * c:OBW * c + S]
                nc.scalar.activation(
                    out=dst, in_=src,
                    func=mybir.ActivationFunctionType.Identity,
                    bias=btile[:mv, :],
                )
                if blk == 0 and c == 0:
                    nc.tensor.matmul(
                        sps[0:2, 4:6],
                        xt[0:2, MARGIN + XSPLIT:MARGIN + XSPLIT + 2],
                        ob[0:2, 0:2], start=True, stop=True)
                last = blk == len(BLOCKS) - 1
                stores = ((4, 0, 2048), (6, 2048, 3072), (8, 3072, 4032)) if last \
                    else ((4, 0, 2048), (8, 2048, 4032))
                for sc, lo, hi in stores:
                    if c != sc:
                        continue
                    osl = out[:, dr0:dr0 + dsc, lo:hi]
                    oap = bass.AP(tensor=osl.tensor, offset=osl.offset,
                                  ap=[osl.ap[1], osl.ap[0], osl.ap[2]])
                    nc.gpsimd.dma_start(out=oap, in_=ob[:mv, lo:hi])
            osl = out[:, dr0:dr0 + dsc, 4032:4096]
            oap = bass.AP(tensor=osl.tensor, offset=osl.offset,
                          ap=[osl.ap[1], osl.ap[0], osl.ap[2]])
            (nc.scalar if blk == len(BLOCKS) - 1 else nc.gpsimd).dma_start(
                out=oap, in_=ob[:mv, 4032:4096])
            prev_ob = ob
    _strip_implied_waits(nc)
    _spread_adjacent_waits(nc)
    _pair_waits_to_ldw(nc)
    _push_waits_earlier(nc)
    _split_tail_drain_waits(nc)
    return nc


def _host_prep(x, weight, bias):
    x = np.ascontiguousarray(x, dtype=np.float32)
    weight = np.ascontiguousarray(weight, dtype=np.float32)
    bias = np.ascontiguousarray(bias, dtype=np.float32)

    xp = np.zeros((B, CIN, 70, PLANE), dtype=_NPDT)
    xpv = xp[:, :, :, :HP * WP].reshape(B, CIN, 70, HP, WP)
    xpv[:, :, 1:S + 1, 1:S + 1, 1:S + 1] = x.astype(_NPDT)

    wbd = np.zeros((CIN, NDW, 9, 96), dtype=_NPDT)
    wt = weight.astype(_NPDT).transpose(1, 0, 2, 3, 4).reshape(CIN, COUT, 3, 9)
    for ds in range(DSUB):
        for kd in range(3):
            wbd[:, ds + kd, :, ds * 16:(ds + 1) * 16] = wt[:, :, kd, :].transpose(0, 2, 1)
    wbd = np.ascontiguousarray(wbd.reshape(128, 9 * 96))

    bias96 = np.ascontiguousarray(np.tile(bias, DSUB)[:, None])

    in_maps = []
    for core in range(8):
        b, h = divmod(core, 2)
        xsh = np.ascontiguousarray(
            xp[b, :, 32 * h:32 * h + SHARD_D].reshape(CIN, SHARD_D, PLANE))
        in_maps.append({"xs": xsh, "wb": wbd, "bs": bias96})
    return in_maps


def kernel(x, weight, bias):
    global _nc_cache, LAST_RESULT
    if _nc_cache is None:
        _nc_cache = _build_nc()
    nc = _nc_cache

    in_maps = _host_prep(x, weight, bias)
    trace = bool(int(os.environ.get("KERNEL_TRACE", "0")))
    res = run_bass_kernel_spmd(nc, in_maps, core_ids=list(range(8)), trace=trace)
    LAST_RESULT = res

    out = np.empty((B, COUT, S, S, S), dtype=np.float32)
    for core in range(8):
        b, h = divmod(core, 2)
        out[b, :, 32 * h:32 * h + 32] = (
            res.results[core]["out"].astype(np.float32).reshape(COUT, DHALF, S, S))
    return out
